# revision 5
# baseline (speedup 1.0000x reference)
"""Trainium2 Bass kernel for nn_MultiHeadAttention_6193342841469.

Module: Q/K/V projections -> per-head attention with additive bias
(conv_w * normal_lps + conv_b) and boolean mask -> fc -> residual ->
LayerNorm.  Returns (out, attn).

Sharding: 8 cores = (batch b in 0..3) x (query half q in 0..1).
Each core handles 512 query rows of one batch for all 16 heads.

Device-side layout choices (everything f32):
  - Q^T_h [64, 512] and K^T_h [64, 1024] are produced directly by the
    projection matmuls (lhsT = W chunk, rhs = X^T chunk); X^T comes
    pre-transposed from the host.
  - scores^T chunk [128 k, 512 q] = K^T_h(lhsT) x Q^T_h(rhs), two heads
    packed via row-tiling (contraction dim is only 64).
  - softmax: unnorm = exp(scores/8) * exp(bias)^T (host precomputes
    exp(bias)^T, with masked entries zeroed); the denominator comes from
    a ones-column appended to V in the PV matmul; attn^T = unnorm * (1/d).
  - context^T [64, 512] per head feeds fc directly as lhsT.
  - fc + residual + LayerNorm in natural layout, written as `out`.
  - attn is written transposed per (head, k-chunk); host untransposes.
"""

import os
import sys
import functools

import numpy as np

for _p in ("/opt/trn_rl_repo", "/root/.axon_site/_ro/trn_rl_repo"):
    if os.path.isdir(_p) and _p not in sys.path:
        sys.path.append(_p)

import concourse.bass as bass
import concourse.mybir as mybir
import concourse.tile as tile
from concourse import bacc
from concourse.bass_utils import run_bass_kernel_spmd

F32 = mybir.dt.float32
F32R = mybir.dt.float32r
AX = mybir.AxisListType
ALU = mybir.AluOpType
ACTF = mybir.ActivationFunctionType

D = 1024          # d_model
S = 1024          # sequence length
R = 512           # query rows per core
H = 16            # heads
DK = 64           # head dim
NP = H // 2       # head pairs
KC = D // 128     # contraction chunks (1024/128)
SC = S // 128     # sequence chunks
LN_EPS = 1e-5
N_CORES = 8


def _build_program(mm_f32r: bool = False):
    MM = F32R if mm_f32r else F32
    nc = bacc.Bacc(
        "TRN2",
        target_bir_lowering=False,
        debug=False,
        enable_asserts=False,
        num_devices=N_CORES,
    )

    # DRAM I/O (per core)
    xqT = nc.dram_tensor("xqT", [D, R], MM, kind="ExternalInput").ap()
    xkT = nc.dram_tensor("xkT", [D, S], MM, kind="ExternalInput").ap()
    xvT = nc.dram_tensor("xvT", [D, S], MM, kind="ExternalInput").ap()
    xq = nc.dram_tensor("xq", [R, D], F32, kind="ExternalInput").ap()
    wq = nc.dram_tensor("wq", [D, D], MM, kind="ExternalInput").ap()
    wk = nc.dram_tensor("wk", [D, D], MM, kind="ExternalInput").ap()
    wv = nc.dram_tensor("wv", [D, D], MM, kind="ExternalInput").ap()
    wfc = nc.dram_tensor("wfc", [D, D], MM, kind="ExternalInput").ap()
    ebT = nc.dram_tensor("ebT", [S, R], F32, kind="ExternalInput").ap()
    attnT = nc.dram_tensor("attnT", [H, S, R], F32, kind="ExternalOutput").ap()
    out = nc.dram_tensor("out", [R, D], F32, kind="ExternalOutput").ap()

    with tile.TileContext(nc) as tc:
        # ---- long-lived SBUF pools ----
        with (
            tc.tile_pool(name="qT", bufs=NP) as p_qT,
            tc.tile_pool(name="kT", bufs=NP) as p_kT,
            tc.tile_pool(name="vv", bufs=SC) as p_v,
            tc.tile_pool(name="ebT", bufs=SC) as p_ebT,
            tc.tile_pool(name="ctx", bufs=NP) as p_ctx,
            tc.tile_pool(name="const", bufs=1) as p_const,
            tc.tile_pool(name="psA", bufs=5, space="PSUM") as ps_a,
            tc.tile_pool(name="psPV", bufs=2, space="PSUM") as ps_pv,
        ):
            ones1 = p_const.tile([1, 128], MM, tag="ones")
            nc.vector.memset(ones1[:], 1.0)
            eps128 = p_const.tile([128, 1], F32, tag="eps")
            nc.vector.memset(eps128[:], LN_EPS)

            qT_t = []   # per pair: [128, 512]  (partitions 0:64 = even head)
            kT_t = []   # per pair: [128, 1024]
            v_t = []    # per k-chunk: [128, 16, 65] (col 64 = ones)
            ebT_t = []  # per k-chunk: [128, 512]
            ctx_t = []  # per pair: [128, 512] context^T

            # ---------------- Q projection ----------------
            with (
                tc.tile_pool(name="wq", bufs=KC) as p_w,
                tc.tile_pool(name="xqT", bufs=KC) as p_x,
            ):
                w_tiles = []
                x_tiles = []
                for c in range(KC):
                    wt = p_w.tile([128, D], MM, tag="w")
                    nc.sync.dma_start(wt[:], wq[c * 128:(c + 1) * 128, :])
                    w_tiles.append(wt)
                    xt = p_x.tile([128, R], MM, tag="x")
                    nc.sync.dma_start(xt[:], xqT[c * 128:(c + 1) * 128, :])
                    x_tiles.append(xt)
                for p in range(NP):
                    ps = ps_a.tile([128, R], F32, tag="a")
                    for c in range(KC):
                        nc.tensor.matmul(
                            ps[:],
                            w_tiles[c][:, p * 128:(p + 1) * 128],
                            x_tiles[c][:],
                            start=(c == 0),
                            stop=(c == KC - 1),
                        )
                    qt = p_qT.tile([128, R], MM, tag="qT")
                    nc.scalar.copy(qt[:], ps[:])
                    qT_t.append(qt)

            # ---------------- K projection ----------------
            with (
                tc.tile_pool(name="wk", bufs=KC) as p_w,
                tc.tile_pool(name="xkT", bufs=KC) as p_x,
            ):
                w_tiles = []
                x_tiles = []
                for c in range(KC):
                    wt = p_w.tile([128, D], MM, tag="w")
                    nc.sync.dma_start(wt[:], wk[c * 128:(c + 1) * 128, :])
                    w_tiles.append(wt)
                    xt = p_x.tile([128, S], MM, tag="x")
                    nc.sync.dma_start(xt[:], xkT[c * 128:(c + 1) * 128, :])
                    x_tiles.append(xt)
                for p in range(NP):
                    kt = p_kT.tile([128, S], MM, tag="kT")
                    for n in range(2):
                        ps = ps_a.tile([128, 512], F32, tag="a")
                        for c in range(KC):
                            nc.tensor.matmul(
                                ps[:],
                                w_tiles[c][:, p * 128:(p + 1) * 128],
                                x_tiles[c][:, n * 512:(n + 1) * 512],
                                start=(c == 0),
                                stop=(c == KC - 1),
                            )
                        nc.scalar.copy(kt[:, n * 512:(n + 1) * 512], ps[:])
                    kT_t.append(kt)

            # ---------------- V projection ----------------
            # V natural: out chunk [128 k-rows, 16*64]; stored strided into
            # [128, 16, 65] with a ones column at [:, :, 64].
            with (
                tc.tile_pool(name="wv", bufs=KC) as p_w,
                tc.tile_pool(name="xvT", bufs=KC) as p_x,
            ):
                w_tiles = []
                x_tiles = []
                for c in range(KC):
                    wt = p_w.tile([128, D], MM, tag="w")
                    nc.sync.dma_start(wt[:], wv[c * 128:(c + 1) * 128, :])
                    w_tiles.append(wt)
                    xt = p_x.tile([128, S], MM, tag="x")
                    nc.sync.dma_start(xt[:], xvT[c * 128:(c + 1) * 128, :])
                    x_tiles.append(xt)
                for kc in range(SC):
                    vt = p_v.tile([128, H, 65], MM, tag="v")
                    nc.vector.memset(vt[:, :, 64:65], 1.0)
                    for n in range(2):
                        ps = ps_a.tile([128, 512], F32, tag="a")
                        for c in range(KC):
                            nc.tensor.matmul(
                                ps[:],
                                x_tiles[c][:, kc * 128:(kc + 1) * 128],
                                w_tiles[c][:, n * 512:(n + 1) * 512],
                                start=(c == 0),
                                stop=(c == KC - 1),
                            )
                        nc.scalar.copy(
                            vt[:, n * 8:(n + 1) * 8, 0:64],
                            ps[:].rearrange("p (h d) -> p h d", d=64),
                        )
                    v_t.append(vt)

            # exp(bias)^T chunks (deferred loads; reused by every head)
            for kc in range(SC):
                et = p_ebT.tile([128, R], F32, tag="ebT")
                nc.sync.dma_start(et[:], ebT[kc * 128:(kc + 1) * 128, :])
                ebT_t.append(et)

            # ---------------- attention ----------------
            with (
                tc.tile_pool(name="un", bufs=2 * SC) as p_un,
                tc.tile_pool(name="attn", bufs=4) as p_attn,
                tc.tile_pool(name="dinv", bufs=4) as p_dinv,
                tc.tile_pool(name="dinvb", bufs=2) as p_dinvb,
            ):
                for p in range(NP):
                    un_tiles = [[], []]
                    for kc in range(SC):
                        for hi in range(2):
                            ps = ps_a.tile([128, 512], F32, tag="a")
                            nc.tensor.matmul(
                                ps[:],
                                kT_t[p][hi * 64:(hi + 1) * 64,
                                        kc * 128:(kc + 1) * 128],
                                qT_t[p][hi * 64:(hi + 1) * 64, :],
                                start=True,
                                stop=True,
                            )
                            un = p_un.tile([128, R], MM, tag="un")
                            # unnorm = exp(scores/8) * exp(bias)^T
                            nc.scalar.activation(un[:], ps[:], ACTF.Exp,
                                                 scale=0.125)
                            nc.vector.tensor_mul(un[:], un[:], ebT_t[kc][:])
                            un_tiles[hi].append(un)
                    for hi in range(2):
                        h = 2 * p + hi
                        psc = ps_pv.tile([65, 512], F32, tag="pv")
                        for kc in range(SC):
                            nc.tensor.matmul(
                                psc[:],
                                v_t[kc][:, h, :],
                                un_tiles[hi][kc][:],
                                start=(kc == 0),
                                stop=(kc == SC - 1),
                            )
                        dinv = p_dinv.tile([1, 512], MM, tag="d")
                        nc.vector.reciprocal(dinv[:], psc[64:65, :])
                        psb = ps_a.tile([128, 512], F32, tag="a")
                        nc.tensor.matmul(psb[:], ones1[:], dinv[:],
                                         start=True, stop=True)
                        dinvb = p_dinvb.tile([128, 512], F32, tag="db")
                        nc.scalar.copy(dinvb[:], psb[:])
                        # context^T normalized -> persistent ctx tile
                        if hi == 0:
                            ct = p_ctx.tile([128, R], MM, tag="ctx")
                            ctx_t.append(ct)
                        nc.vector.tensor_mul(
                            ctx_t[p][hi * 64:(hi + 1) * 64, :],
                            psc[0:64, :],
                            dinvb[0:64, :],
                        )
                        for kc in range(SC):
                            at = p_attn.tile([128, R], F32, tag="at")
                            nc.vector.tensor_mul(at[:], un_tiles[hi][kc][:],
                                                 dinvb[:])
                            nc.sync.dma_start(
                                attnT[h, kc * 128:(kc + 1) * 128, :], at[:])

            # ---------------- fc + residual + layernorm ----------------
            with (
                tc.tile_pool(name="wfc", bufs=KC) as p_w,
                tc.tile_pool(name="xqr", bufs=4) as p_xq,
                tc.tile_pool(name="ln", bufs=2) as p_ln,
                tc.tile_pool(name="lnsc", bufs=2) as p_lnsc,
                tc.tile_pool(name="stats", bufs=8) as p_st,
                tc.tile_pool(name="outt", bufs=2) as p_out,
            ):
                w_tiles = []
                for c in range(KC):
                    wt = p_w.tile([128, D], MM, tag="w")
                    nc.sync.dma_start(wt[:], wfc[c * 128:(c + 1) * 128, :])
                    w_tiles.append(wt)
                for qc in range(4):
                    xqt = p_xq.tile([128, D], F32, tag="xq")
                    nc.sync.dma_start(xqt[:], xq[qc * 128:(qc + 1) * 128, :])
                    pre = p_ln.tile([128, D], F32, tag="pre")
                    for n in range(2):
                        ps = ps_a.tile([128, 512], F32, tag="a")
                        for p in range(NP):
                            nc.tensor.matmul(
                                ps[:],
                                ctx_t[p][:, qc * 128:(qc + 1) * 128],
                                w_tiles[p][:, n * 512:(n + 1) * 512],
                                start=(p == 0),
                                stop=(p == NP - 1),
                            )
                        nc.vector.tensor_add(pre[:, n * 512:(n + 1) * 512],
                                             ps[:],
                                             xqt[:, n * 512:(n + 1) * 512])
                    sum_ = p_st.tile([128, 1], F32, tag="sum")
                    nc.vector.tensor_reduce(sum_[:], pre[:], axis=AX.X,
                                            op=ALU.add)
                    mu = p_st.tile([128, 1], F32, tag="mu")
                    nc.vector.tensor_scalar_mul(mu[:], sum_[:], 1.0 / D)
                    scr = p_lnsc.tile([128, D], F32, tag="scr")
                    varsum = p_st.tile([128, 1], F32, tag="vs")
                    # sum((x - mu) * x) = D * var
                    nc.vector.scalar_tensor_tensor(
                        scr[:], pre[:], mu[:], pre[:],
                        op0=ALU.subtract, op1=ALU.mult, accum_out=varsum[:])
                    std = p_st.tile([128, 1], F32, tag="std")
                    nc.scalar.activation(std[:], varsum[:], ACTF.Sqrt,
                                         bias=eps128[:], scale=1.0 / D)
                    rstd = p_st.tile([128, 1], F32, tag="rstd")
                    nc.vector.reciprocal(rstd[:], std[:])
                    ot = p_out.tile([128, D], F32, tag="out")
                    nc.vector.tensor_scalar(ot[:], pre[:], mu[:], rstd[:],
                                            op0=ALU.subtract, op1=ALU.mult)
                    nc.sync.dma_start(out[qc * 128:(qc + 1) * 128, :], ot[:])

    nc.compile()
    return nc


@functools.lru_cache(maxsize=2)
def _get_program():
    return _build_program(
        mm_f32r=os.environ.get("KERNEL_MM_F32R", "0") == "1")


def _host_prep(input_Q, input_K, input_V, attn_mask, normal_lps,
               W_Q, W_K, W_V, W_fc, conv_w, conv_b):
    """Build the 8 per-core input maps (sharding + layout prep only)."""
    f = np.float32
    input_Q = np.asarray(input_Q, f)
    input_K = np.asarray(input_K, f)
    input_V = np.asarray(input_V, f)
    attn_mask = np.asarray(attn_mask, bool)
    normal_lps = np.asarray(normal_lps, f)
    W_Q = np.ascontiguousarray(np.asarray(W_Q, f))
    W_K = np.ascontiguousarray(np.asarray(W_K, f))
    W_V = np.ascontiguousarray(np.asarray(W_V, f))
    W_fc = np.ascontiguousarray(np.asarray(W_fc, f))

    bias = (f(conv_w) * normal_lps + f(conv_b)).astype(f)  # [S, S] (q, k)
    eb = np.exp(bias)

    in_maps = []
    for core in range(N_CORES):
        b, qh = divmod(core, 2)
        q0 = qh * R
        xT_q = np.ascontiguousarray(input_Q[b].T)  # [D, S]
        ebT_b = eb.T.copy()  # [k, q]
        m = attn_mask[b]
        if m.any():
            ebT_b[m.T] = 0.0
        in_maps.append({
            "xqT": np.ascontiguousarray(xT_q[:, q0:q0 + R]),
            "xkT": np.ascontiguousarray(input_K[b].T),
            "xvT": np.ascontiguousarray(input_V[b].T),
            "xq": np.ascontiguousarray(input_Q[b][q0:q0 + R]),
            "wq": W_Q, "wk": W_K, "wv": W_V, "wfc": W_fc,
            "ebT": np.ascontiguousarray(ebT_b[:, q0:q0 + R]),
        })
    return in_maps


def _assemble(results):
    B = 4
    out = np.empty((B, S, D), np.float32)
    attn = np.empty((B, H, S, S), np.float32)
    for core in range(N_CORES):
        b, qh = divmod(core, 2)
        q0 = qh * R
        r = results[core]
        out[b, q0:q0 + R, :] = r["out"]
        attn[b, :, q0:q0 + R, :] = r["attnT"].transpose(0, 2, 1)
    return out, attn


def _ensure_ntff_hook():
    """The agent image's antenv lacks axon_hooks; synthesize it and install
    the ctypes NTFF profiling hook from trn_agent_boot."""
    try:
        from antenv import axon_hooks  # noqa: F401
        return
    except ImportError:
        pass
    import types
    import antenv
    mod = types.ModuleType("antenv.axon_hooks")
    _h = {"hook": None}
    mod.set_axon_ntff_profile_hook = lambda h: _h.__setitem__("hook", h)
    mod.get_axon_ntff_profile_hook = lambda: _h["hook"]
    sys.modules["antenv.axon_hooks"] = mod
    antenv.axon_hooks = mod
    try:
        from trn_agent_boot.trn_boot import _ntff_profile_via_ctypes
        mod.set_axon_ntff_profile_hook(
            _ntff_profile_via_ctypes("/opt/axon/libaxon_pjrt.so"))
    except Exception as e:  # profiling degrades; execution still works
        print("ntff hook install failed:", e, file=sys.stderr)


def run(inputs: dict, trace: bool = False):
    if trace:
        _ensure_ntff_hook()
    nc = _get_program()
    in_maps = _host_prep(**inputs)
    res = run_bass_kernel_spmd(nc, in_maps, list(range(N_CORES)), trace=trace)
    out, attn = _assemble(res.results)
    return (out, attn), res


def kernel(**inputs):
    (out, attn), _ = run(inputs, trace=False)
    return out, attn


# revision 13
# speedup vs baseline: 1.6892x; 1.6892x over previous
"""Trainium2 Bass kernel for nn_MultiHeadAttention_6193342841469.

Module: Q/K/V projections -> per-head attention with additive bias
(conv_w * normal_lps + conv_b) and boolean mask -> fc -> residual ->
LayerNorm.  Returns (out, attn).

Sharding: 8 cores = (batch b in 0..3) x (query half q in 0..1).
Each core handles 512 query rows of one batch for all 16 heads.

Device-side layout choices (everything f32):
  - Q^T_h [64, 512] and K^T_h [64, 1024] are produced directly by the
    projection matmuls (lhsT = W chunk, rhs = X^T chunk); X^T comes
    pre-transposed from the host.
  - scores^T chunk [128 k, 512 q] = K^T_h(lhsT) x Q^T_h(rhs), two heads
    packed via row-tiling (contraction dim is only 64).
  - softmax: unnorm = exp(scores/8) * exp(bias)^T (host precomputes
    exp(bias)^T, with masked entries zeroed); the denominator comes from
    a ones-column appended to V in the PV matmul; attn^T = unnorm * (1/d).
  - context^T [64, 512] per head feeds fc directly as lhsT.
  - fc + residual + LayerNorm in natural layout, written as `out`.
  - attn is written transposed per (head, k-chunk); host untransposes.
"""

import os
import sys
import functools

import numpy as np

for _p in ("/opt/trn_rl_repo", "/root/.axon_site/_ro/trn_rl_repo"):
    if os.path.isdir(_p) and _p not in sys.path:
        sys.path.append(_p)

import concourse.bass as bass
import concourse.mybir as mybir
import concourse.tile as tile
from concourse import bacc
from concourse.bass_utils import run_bass_kernel_spmd

F32 = mybir.dt.float32
F32R = mybir.dt.float32r
AX = mybir.AxisListType
ALU = mybir.AluOpType
ACTF = mybir.ActivationFunctionType

D = 1024          # d_model
S = 1024          # sequence length
R = 512           # query rows per core
H = 16            # heads
DK = 64           # head dim
NP = H // 2       # head pairs
KC = D // 128     # contraction chunks (1024/128)
SC = S // 128     # sequence chunks
LN_EPS = 1e-5
N_CORES = 8


def _build_program(mm_f32r: bool = False):
    MM = F32R if mm_f32r else F32
    nc = bacc.Bacc(
        "TRN2",
        target_bir_lowering=False,
        debug=False,
        enable_asserts=False,
        num_devices=N_CORES,
    )

    # DRAM I/O (per core)
    xqT = nc.dram_tensor("xqT", [D, R], MM, kind="ExternalInput").ap()
    xkT = nc.dram_tensor("xkT", [D, S], MM, kind="ExternalInput").ap()
    xvT = nc.dram_tensor("xvT", [D, S], MM, kind="ExternalInput").ap()
    xq = nc.dram_tensor("xq", [R, D], F32, kind="ExternalInput").ap()
    wq = nc.dram_tensor("wq", [D, D], MM, kind="ExternalInput").ap()
    wk = nc.dram_tensor("wk", [D, D], MM, kind="ExternalInput").ap()
    wv = nc.dram_tensor("wv", [D, D], MM, kind="ExternalInput").ap()
    wfc = nc.dram_tensor("wfc", [D, D], MM, kind="ExternalInput").ap()
    ebT = nc.dram_tensor("ebT", [S, R], F32, kind="ExternalInput").ap()
    vones = nc.dram_tensor("vones", [128, H], MM, kind="ExternalInput").ap()
    attnT = nc.dram_tensor("attnT", [H, S, R], F32, kind="ExternalOutput").ap()
    out = nc.dram_tensor("out", [R, D], F32, kind="ExternalOutput").ap()

    import contextlib
    lp = (nc.allow_low_precision(reason="float32r matmul experiment")
          if mm_f32r else contextlib.nullcontext())
    with lp, tile.TileContext(nc) as tc:
        # ---- long-lived SBUF pools ----
        with (
            tc.tile_pool(name="qT", bufs=NP) as p_qT,
            tc.tile_pool(name="kT", bufs=NP) as p_kT,
            tc.tile_pool(name="vv", bufs=SC) as p_v,
            tc.tile_pool(name="ebT", bufs=SC) as p_ebT,
            tc.tile_pool(name="ctx", bufs=NP) as p_ctx,
            tc.tile_pool(name="const", bufs=1) as p_const,
            tc.tile_pool(name="psA", bufs=5, space="PSUM") as ps_a,
            tc.tile_pool(name="psPV", bufs=2, space="PSUM") as ps_pv,
        ):
            ones1 = p_const.tile([1, 128], F32, tag="ones")
            nc.vector.memset(ones1[:], 1.0)
            eps128 = p_const.tile([128, 1], F32, tag="eps")
            nc.vector.memset(eps128[:], LN_EPS)

            qT_t = []   # per pair: [128, 512]  (partitions 0:64 = even head)
            kT_t = []   # per pair: [128, 1024]
            v_t = []    # per k-chunk: [128, 16, 65] (col 64 = ones)
            ebT_t = []  # per k-chunk: [128, 512]
            ctx_t = []  # per pair: [128, 512] context^T

            # ---------------- Q projection ----------------
            with (
                tc.tile_pool(name="wq", bufs=KC) as p_w,
                tc.tile_pool(name="xqT", bufs=KC) as p_x,
            ):
                w_tiles = []
                x_tiles = []
                for c in range(KC):
                    wt = p_w.tile([128, D], MM, tag="w")
                    nc.sync.dma_start(wt[:], wq[c * 128:(c + 1) * 128, :])
                    w_tiles.append(wt)
                    xt = p_x.tile([128, R], MM, tag="x")
                    nc.sync.dma_start(xt[:], xqT[c * 128:(c + 1) * 128, :])
                    x_tiles.append(xt)
                for p in range(NP):
                    ps = ps_a.tile([128, R], F32, tag="a")
                    for c in range(KC):
                        nc.tensor.matmul(
                            ps[:],
                            (w_tiles[c][:, p * 128:(p + 1) * 128]),
                            (x_tiles[c][:]),
                            start=(c == 0),
                            stop=(c == KC - 1),
                        )
                    qt = p_qT.tile([128, R], MM, tag="qT")
                    nc.scalar.copy(qt[:], ps[:])
                    qT_t.append(qt)

            # ---------------- K projection ----------------
            with (
                tc.tile_pool(name="wk", bufs=KC) as p_w,
                tc.tile_pool(name="xkT", bufs=KC) as p_x,
            ):
                w_tiles = []
                x_tiles = []
                for c in range(KC):
                    wt = p_w.tile([128, D], MM, tag="w")
                    nc.sync.dma_start(wt[:], wk[c * 128:(c + 1) * 128, :])
                    w_tiles.append(wt)
                    xt = p_x.tile([128, S], MM, tag="x")
                    nc.sync.dma_start(xt[:], xkT[c * 128:(c + 1) * 128, :])
                    x_tiles.append(xt)
                for p in range(NP):
                    kt = p_kT.tile([128, S], MM, tag="kT")
                    for n in range(2):
                        ps = ps_a.tile([128, 512], F32, tag="a")
                        for c in range(KC):
                            nc.tensor.matmul(
                                ps[:],
                                (w_tiles[c][:, p * 128:(p + 1) * 128]),
                                (x_tiles[c][:, n * 512:(n + 1) * 512]),
                                start=(c == 0),
                                stop=(c == KC - 1),
                            )
                        nc.scalar.copy(kt[:, n * 512:(n + 1) * 512], ps[:])
                    kT_t.append(kt)

            # ---------------- V projection ----------------
            # V natural: out chunk [128 k-rows, 16*64]; stored strided into
            # [128, 16, 65] with a ones column at [:, :, 64].
            with (
                tc.tile_pool(name="wv", bufs=KC) as p_w,
                tc.tile_pool(name="xvT", bufs=KC) as p_x,
            ):
                w_tiles = []
                x_tiles = []
                for c in range(KC):
                    wt = p_w.tile([128, D], MM, tag="w")
                    nc.sync.dma_start(wt[:], wv[c * 128:(c + 1) * 128, :])
                    w_tiles.append(wt)
                    xt = p_x.tile([128, S], MM, tag="x")
                    nc.sync.dma_start(xt[:], xvT[c * 128:(c + 1) * 128, :])
                    x_tiles.append(xt)
                for kc in range(SC):
                    vt = p_v.tile([128, H, 65], MM, tag="v")
                    nc.sync.dma_start(
                        vt[:, :, 64:65],
                        vones[:].rearrange("p (h o) -> p h o", o=1))
                    for n in range(2):
                        ps = ps_a.tile([128, 512], F32, tag="a")
                        for c in range(KC):
                            nc.tensor.matmul(
                                ps[:],
                                (x_tiles[c][:, kc * 128:(kc + 1) * 128]),
                                (w_tiles[c][:, n * 512:(n + 1) * 512]),
                                start=(c == 0),
                                stop=(c == KC - 1),
                            )
                        nc.scalar.copy(
                            vt[:, n * 8:(n + 1) * 8, 0:64],
                            ps[:].rearrange("p (h d) -> p h d", d=64),
                        )
                    v_t.append(vt)

            # exp(bias)^T chunks (deferred loads; reused by every head)
            for kc in range(SC):
                et = p_ebT.tile([128, R], F32, tag="ebT")
                nc.sync.dma_start(et[:], ebT[kc * 128:(kc + 1) * 128, :])
                ebT_t.append(et)

            # ---------------- attention ----------------
            with (
                tc.tile_pool(name="un", bufs=4 * SC) as p_un,
                tc.tile_pool(name="attn", bufs=4) as p_attn,
                tc.tile_pool(name="dinv", bufs=4) as p_dinv,
                tc.tile_pool(name="dinvb", bufs=2) as p_dinvb,
            ):
                pair_un = {}

                def emit_qk(p):
                    un_tiles = [[], []]
                    for kc in range(SC):
                        for hi in range(2):
                            ps = ps_a.tile([128, 512], F32, tag="a")
                            nc.tensor.matmul(
                                ps[:],
                                (kT_t[p][hi * 64:(hi + 1) * 64,
                                              kc * 128:(kc + 1) * 128]),
                                (qT_t[p][hi * 64:(hi + 1) * 64, :]),
                                start=True,
                                stop=True,
                            )
                            un = p_un.tile([128, R], MM, tag="un")
                            # unnorm = exp(scores/8) * exp(bias)^T
                            nc.scalar.activation(un[:], ps[:], ACTF.Exp,
                                                 scale=0.125)
                            nc.vector.tensor_mul(un[:], un[:], ebT_t[kc][:])
                            un_tiles[hi].append(un)
                    pair_un[p] = un_tiles

                def emit_tail(p):
                    un_tiles = pair_un.pop(p)
                    for hi in range(2):
                        h = 2 * p + hi
                        psc = ps_pv.tile([65, 512], F32, tag="pv")
                        for kc in range(SC):
                            nc.tensor.matmul(
                                psc[:],
                                (v_t[kc][:, h, :]),
                                (un_tiles[hi][kc][:]),
                                start=(kc == 0),
                                stop=(kc == SC - 1),
                            )
                        dinv = p_dinv.tile([1, 512], F32, tag="d")
                        nc.vector.reciprocal_approx_fast(dinv[:], psc[64:65, :])
                        psb = ps_a.tile([128, 512], F32, tag="a")
                        nc.tensor.matmul(psb[:], ones1[:], dinv[:],
                                         start=True, stop=True)
                        dinvb = p_dinvb.tile([128, 512], F32, tag="db")
                        nc.scalar.copy(dinvb[:], psb[:])
                        # context^T normalized -> persistent ctx tile
                        if hi == 0:
                            ct = p_ctx.tile([128, R], MM, tag="ctx")
                            ctx_t.append(ct)
                        nc.vector.tensor_mul(
                            ctx_t[p][hi * 64:(hi + 1) * 64, :],
                            psc[0:64, :],
                            dinvb[0:64, :],
                        )
                        for kc in range(SC):
                            at = p_attn.tile([128, R], F32, tag="at")
                            nc.vector.tensor_mul(at[:], un_tiles[hi][kc][:],
                                                 dinvb[:])
                            nc.sync.dma_start(
                                attnT[h, kc * 128:(kc + 1) * 128, :], at[:])

                # software pipeline: pair p's PV/normalize tail is emitted
                # after pair p+1's QK, so the PE never waits on the
                # exp/mul/reciprocal chain.
                for p in range(NP):
                    emit_qk(p)
                    if p >= 1:
                        emit_tail(p - 1)
                emit_tail(NP - 1)

            # ---------------- fc + residual + layernorm ----------------
            with (
                tc.tile_pool(name="wfc", bufs=KC) as p_w,
                tc.tile_pool(name="xqr", bufs=4) as p_xq,
                tc.tile_pool(name="ln", bufs=2) as p_ln,
                tc.tile_pool(name="lnsc", bufs=2) as p_lnsc,
                tc.tile_pool(name="stats", bufs=8) as p_st,
                tc.tile_pool(name="outt", bufs=2) as p_out,
            ):
                w_tiles = []
                for c in range(KC):
                    wt = p_w.tile([128, D], MM, tag="w")
                    nc.sync.dma_start(wt[:], wfc[c * 128:(c + 1) * 128, :])
                    w_tiles.append(wt)
                for qc in range(4):
                    xqt = p_xq.tile([128, D], F32, tag="xq")
                    nc.sync.dma_start(xqt[:], xq[qc * 128:(qc + 1) * 128, :])
                    pre = p_ln.tile([128, D], F32, tag="pre")
                    for n in range(2):
                        ps = ps_a.tile([128, 512], F32, tag="a")
                        for p in range(NP):
                            nc.tensor.matmul(
                                ps[:],
                                (ctx_t[p][:, qc * 128:(qc + 1) * 128]),
                                (w_tiles[p][:, n * 512:(n + 1) * 512]),
                                start=(p == 0),
                                stop=(p == NP - 1),
                            )
                        nc.vector.tensor_add(pre[:, n * 512:(n + 1) * 512],
                                             ps[:],
                                             xqt[:, n * 512:(n + 1) * 512])
                    sum_ = p_st.tile([128, 1], F32, tag="sum")
                    nc.vector.tensor_reduce(sum_[:], pre[:], axis=AX.X,
                                            op=ALU.add)
                    mu = p_st.tile([128, 1], F32, tag="mu")
                    nc.vector.tensor_scalar_mul(mu[:], sum_[:], 1.0 / D)
                    scr = p_lnsc.tile([128, D], F32, tag="scr")
                    varsum = p_st.tile([128, 1], F32, tag="vs")
                    # sum((x - mu) * x) = D * var
                    nc.vector.scalar_tensor_tensor(
                        scr[:], pre[:], mu[:], pre[:],
                        op0=ALU.subtract, op1=ALU.mult, accum_out=varsum[:])
                    std = p_st.tile([128, 1], F32, tag="std")
                    nc.scalar.activation(std[:], varsum[:], ACTF.Sqrt,
                                         bias=eps128[:], scale=1.0 / D)
                    rstd = p_st.tile([128, 1], F32, tag="rstd")
                    nc.vector.reciprocal(rstd[:], std[:])
                    ot = p_out.tile([128, D], F32, tag="out")
                    nc.vector.tensor_scalar(ot[:], pre[:], mu[:], rstd[:],
                                            op0=ALU.subtract, op1=ALU.mult)
                    nc.sync.dma_start(out[qc * 128:(qc + 1) * 128, :], ot[:])

    nc.compile()
    return nc


@functools.lru_cache(maxsize=2)
def _get_program():
    return _build_program(
        mm_f32r=os.environ.get("KERNEL_MM_F32R", "0") == "1")


def _host_prep(input_Q, input_K, input_V, attn_mask, normal_lps,
               W_Q, W_K, W_V, W_fc, conv_w, conv_b):
    """Build the 8 per-core input maps (sharding + layout prep only)."""
    f = np.float32
    input_Q = np.asarray(input_Q, f)
    input_K = np.asarray(input_K, f)
    input_V = np.asarray(input_V, f)
    attn_mask = np.asarray(attn_mask, bool)
    normal_lps = np.asarray(normal_lps, f)
    W_Q = np.ascontiguousarray(np.asarray(W_Q, f))
    W_K = np.ascontiguousarray(np.asarray(W_K, f))
    W_V = np.ascontiguousarray(np.asarray(W_V, f))
    W_fc = np.ascontiguousarray(np.asarray(W_fc, f))

    bias = (f(conv_w) * normal_lps + f(conv_b)).astype(f)  # [S, S] (q, k)
    eb = np.exp(bias)

    in_maps = []
    for core in range(N_CORES):
        b, qh = divmod(core, 2)
        q0 = qh * R
        xT_q = np.ascontiguousarray(input_Q[b].T)  # [D, S]
        ebT_b = eb.T.copy()  # [k, q]
        m = attn_mask[b]
        if m.any():
            ebT_b[m.T] = 0.0
        in_maps.append({
            "xqT": np.ascontiguousarray(xT_q[:, q0:q0 + R]),
            "xkT": np.ascontiguousarray(input_K[b].T),
            "xvT": np.ascontiguousarray(input_V[b].T),
            "xq": np.ascontiguousarray(input_Q[b][q0:q0 + R]),
            "wq": W_Q, "wk": W_K, "wv": W_V, "wfc": W_fc,
            "vones": np.ones((128, H), f),
            "ebT": np.ascontiguousarray(ebT_b[:, q0:q0 + R]),
        })
    return in_maps


def _assemble(results):
    B = 4
    out = np.empty((B, S, D), np.float32)
    attn = np.empty((B, H, S, S), np.float32)
    for core in range(N_CORES):
        b, qh = divmod(core, 2)
        q0 = qh * R
        r = results[core]
        out[b, q0:q0 + R, :] = r["out"]
        attn[b, :, q0:q0 + R, :] = r["attnT"].transpose(0, 2, 1)
    return out, attn


def _ensure_ntff_hook():
    """The agent image's antenv lacks axon_hooks; synthesize it and install
    the ctypes NTFF profiling hook from trn_agent_boot."""
    try:
        from antenv import axon_hooks  # noqa: F401
        return
    except ImportError:
        pass
    import types
    import antenv
    mod = types.ModuleType("antenv.axon_hooks")
    _h = {"hook": None}
    mod.set_axon_ntff_profile_hook = lambda h: _h.__setitem__("hook", h)
    mod.get_axon_ntff_profile_hook = lambda: _h["hook"]
    sys.modules["antenv.axon_hooks"] = mod
    antenv.axon_hooks = mod
    try:
        from trn_agent_boot.trn_boot import _ntff_profile_via_ctypes
        mod.set_axon_ntff_profile_hook(
            _ntff_profile_via_ctypes("/opt/axon/libaxon_pjrt.so"))
    except Exception as e:  # profiling degrades; execution still works
        print("ntff hook install failed:", e, file=sys.stderr)


def run(inputs: dict, trace: bool = False):
    if trace:
        _ensure_ntff_hook()
    nc = _get_program()
    in_maps = _host_prep(**inputs)
    res = run_bass_kernel_spmd(nc, in_maps, list(range(N_CORES)), trace=trace)
    out, attn = _assemble(res.results)
    return (out, attn), res


def kernel(**inputs):
    (out, attn), _ = run(inputs, trace=False)
    return out, attn
